# Initial kernel scaffold
#
"""Trainium2 Bass kernel for nn_DifferentiableEditLayer.

Strategy (per core = one batch sample, pure data parallel across 8 cores):
  - All per-sample scalar params precomputed on host, passed as a [128, NP]
    broadcast tensor.
  - The 256-point tone curve is interpolated to the 1024-point curve on host
    and baked into a CUSTOM ScalarEngine (ACT) piecewise-cubic table as the
    RATIO function f(v) = target(v/1023)/max(v/1023, 1e-5) over v in [0,1023],
    one table set per core (8 hijacked activation-function names), evaluated
    at line rate by a single ACTIVATE per tile. Additional custom ACT
    functions: recip4(x)=1/max(x,1e-4) (hijacks 'ln') and
    expsqrt(s)=exp(-4*sqrt(s)) (8 per-core names), plus a copied stock
    sigmoid, so the whole pipeline needs no table switching within a phase.
  - Phase 1 streams the image from HBM, computes luma1 (after white balance /
    exposure / contrast) into an SBUF-resident plane and accumulates the
    sigmoid sum for the first region mean.
  - The 4 region ("highlights/shadows/whites/blacks") updates run entirely in
    SBUF on the luma plane; per-pixel ratio products P = r1*r2*r3*r4 and
    S = min(1, r4*min(1, r3*min(1, r2))) (== min(R2,R3,R4,1)) are maintained
    as planes.
  - The final pass re-streams the image, recomputes the white-balance stage,
    applies min(img*P, S), tone-curve ratio, vibrance and saturation, and
    writes the output.
"""
import os, json, struct, shutil, hashlib, tempfile
import numpy as np

# ----------------------------------------------------------------------------
# constants
# ----------------------------------------------------------------------------
B, C, H, W = 8, 3, 1024, 1536
NPIX = H * W            # 1,572,864
P = 128                 # SBUF partitions
FREE = NPIX // P        # 12288
F = 1024                # chunk free size
NCH = FREE // F         # 12 chunks

TC_NAMES = ["sin", "arctan", "erf", "gelu", "silu", "derivative_silu",
            "gelu_apprx_tanh", "derivative_gelu"]
ES_NAMES = ["exp"] * 8
RECIP4_NAME = "ln"
FILLERS = ["identity", "copy", "act1", "relu", "abs", "sign",
           "memset_zero", "derivative_relu",
           "derivative_identity", "square"]

# region (pivot, width) and derived sigmoid affine (scale, bias), compile-time
REGIONS = [(0.7, 0.1), (0.3, 0.12), (0.9, 0.08), (0.1, 0.08)]
SIG_AFF = [(1.0 / w, -p / w) for (p, w) in REGIONS]

# prm layout
PRM = dict(A_r=0, A_g=1, A_b=2, t=3, hi=4, s1=5, s2=6, s3=7, s4=8,
           v=9, gs=10, omgs=11, invN=12, b1=13, b2=14, b3=15, b4=16, esb=17)
NP_ = 18


# ----------------------------------------------------------------------------
# custom ACT table generation
# ----------------------------------------------------------------------------

def _stock_dir():
    import neuronxcc
    return os.path.join(os.path.dirname(neuronxcc.__file__), "pwp", "pwp_bin_trainium")


def _load_set(name):
    d = _stock_dir()
    j = json.load(open(f"{d}/{name}.json"))
    ctrl = open(f"{d}/{name}_ctrl.bin", "rb").read()
    bkt = open(f"{d}/{name}_bkt.bin", "rb").read()
    return j, ctrl, bkt


def _func_span(j, fname, kind):
    key = "func_to_bkt_start_idx" if kind == "b" else "func_to_ctl_start_idx"
    cnt = j["bkt_entry_cnt"] if kind == "b" else j["ctl_entry_cnt"]
    starts = j[key]
    s = starts[fname]
    nxt = [v for v in starts.values() if v > s]
    return s, (min(nxt) if nxt else cnt)


class _SetBuilder:
    def __init__(self, name):
        self.name = name
        self.ctl, self.bkt, self.profile = [], [], []
        self.f2b, self.f2c, self.fe2b, self.fe2c, self.act = {}, {}, {}, {}, {}

    def copy_stock_func(self, set_json, ctrl_bin, bkt_bin, fname, ulp):
        b0, b1 = _func_span(set_json, fname, "b")
        c0, c1 = _func_span(set_json, fname, "c")
        boff = len(self.bkt) - b0
        coff = len(self.ctl) - c0
        for i in range(b0, b1):
            self.bkt.append(struct.unpack_from("<5f", bkt_bin, i * 32))
        for i in range(c0, c1):
            d = struct.unpack_from("<I", ctrl_bin, i * 32)[0]
            self.ctl.append((d & ~0x7FF) | (((d & 0x7FF) + boff) & 0x7FF))
        ent = None
        for e in set_json["profile_meta_data"]:
            nm = e["func_name"]
            if nm == fname or nm.rsplit("_", 1)[0] == fname or nm.startswith(fname + "_"):
                ent = dict(e)
                break
        assert ent is not None, f"no profile entry for {fname}"
        for k in ("pwl_control_base_pos", "pwl_control_base_neg"):
            ent[k] = ent.get(k, 0) + coff
        for k in ("pos_small_signal_pwl_control", "neg_small_signal_pwl_control",
                  "pos_large_signal_pwl_control", "neg_large_signal_pwl_control"):
            ent[k] = ent.get(k, 0) + boff
        self.profile.append(ent)
        self.f2b[fname] = b0 + boff
        self.f2c[fname] = c0 + coff
        self.fe2b[fname] = {k: [v + boff for v in vs] for k, vs in set_json["func_exp_to_bkt_start_idx"].get(fname, {}).items()}
        self.fe2c[fname] = {k: [v + coff for v in vs] for k, vs in set_json["func_exp_to_ctl_start_idx"].get(fname, {}).items()}
        self.act[fname] = ulp

    def add_pwp_func(self, fname, func_id, octaves, fit_fn, fzero, small_const,
                     large_const, template_entry, ulp=4):
        bstart, cstart = len(self.bkt), len(self.ctl)
        fe2b, fe2c = {}, {}
        for (e, nb) in octaves:
            n = 1 << nb
            lo_oct = float(2.0 ** e)
            w = lo_oct / n
            fe2c[str(e)] = [len(self.ctl)]
            fe2b[str(e)] = [len(self.bkt)]
            self.ctl.append((len(self.bkt) & 0x7FF) | ((23 - nb) << 11) | (nb << 16))
            for i in range(n):
                lo = lo_oct + i * w
                d0, d1, d2, d3 = fit_fn(lo, lo + w)
                self.bkt.append((d0, d1, d2, d3, np.float32(lo)))
        small_bkt = len(self.bkt)
        self.bkt.append((small_const, 0.0, 0.0, 0.0, 0.0))
        large_bkt = len(self.bkt)
        self.bkt.append((large_const, 0.0, 0.0, 0.0, 0.0))
        e_lo, e_hi = octaves[0][0], octaves[-1][0]
        ent = dict(template_entry)
        ent.update(func_name=fname + "_4p", func_id=func_id, symmetry_point=0,
                   sym_invert_sign_point=0, symmetry_opt_en=0,
                   symmetry_opt_use_neg_region=0, imm_bias=0, exp_offset=e_lo,
                   pwl_control_base_pos=cstart, pwl_control_base_neg=cstart,
                   small_pos_signal_exp_threshold=e_lo + 127,
                   pos_small_signal_pwl_control=small_bkt,
                   small_neg_signal_exp_threshold=0,
                   neg_small_signal_pwl_control=small_bkt,
                   large_pos_signal_exp_threshold=e_hi + 1 + 127,
                   large_pos_signal_mantissa_threshold=0,
                   pos_large_signal_pwl_control=large_bkt,
                   large_neg_signal_exp_threshold=0,
                   large_neg_signal_mantissa_threshold=0,
                   neg_large_signal_pwl_control=small_bkt,
                   fzero_result=int(np.float32(fzero).view(np.uint32)),
                   fnan_result=int(np.float32(fzero).view(np.uint32)),
                   fpinf_result=int(np.float32(large_const).view(np.uint32)),
                   fninf_result=int(np.float32(small_const).view(np.uint32)),
                   fma_const_0=0, fma_const_1=0, fma_indirection_src_sel=0,
                   use_multipass=False,
                   lower_bound=int(np.float32(2.0 ** e_lo).view(np.uint32)),
                   upper_bound=int(np.float32(2.0 ** (e_hi + 1)).view(np.uint32)))
        self.profile.append(ent)
        self.f2b[fname], self.f2c[fname] = bstart, cstart
        self.fe2b[fname], self.fe2c[fname] = fe2b, fe2c
        self.act[fname] = ulp

    def finalize(self, outdir):
        assert len(self.bkt) <= 1536, f"{self.name}: {len(self.bkt)} buckets"
        j = {"bkt_bin": f"{self.name}_bkt.bin", "ctl_bin": f"{self.name}_ctrl.bin",
             "profile_meta_data": self.profile,
             "bkt_entry_cnt": len(self.bkt), "ctl_entry_cnt": len(self.ctl),
             "func_to_bkt_start_idx": self.f2b, "func_to_ctl_start_idx": self.f2c,
             "func_exp_to_bkt_start_idx": self.fe2b,
             "func_exp_to_ctl_start_idx": self.fe2c}
        json.dump(j, open(f"{outdir}/{self.name}.json", "w"))
        with open(f"{outdir}/{self.name}_ctrl.bin", "wb") as f:
            for d in self.ctl:
                f.write(struct.pack("<I", d) + b"\0" * 28)
        with open(f"{outdir}/{self.name}_bkt.bin", "wb") as f:
            for b in self.bkt:
                f.write(struct.pack("<5f", *b) + b"\0" * 12)
        return {"name": self.name, "bkt_bin": j["bkt_bin"], "ctrl_bin": j["ctl_bin"],
                "profile_json": f"{self.name}.json", "act": self.act}


def _fit_cubic(fn, lo, hi, M=9):
    xs = np.linspace(lo, hi, M, dtype=np.float64)
    t = xs - lo
    A = np.stack([np.ones_like(t), t, t * t, t ** 3], axis=1)
    c, *_ = np.linalg.lstsq(A, fn(xs), rcond=None)
    return tuple(np.float32(v) for v in c)


def _make_ratio_fit(curve1024):
    c = np.asarray(curve1024, np.float64)
    vstar = 1023.0e-5

    def g(v):
        v = np.asarray(v, np.float64)
        i = np.clip(np.floor(v).astype(int), 0, 1022)
        w = v - i
        tgt = c[i] * (1 - w) + c[i + 1] * w
        tgt = np.where(v >= 1023, c[1023], tgt)
        return tgt * 1023.0 / np.maximum(v, vstar)

    def fit(lo, hi):
        if hi <= vstar:
            return (np.float32(1.0), np.float32(0), np.float32(0), np.float32(0))
        lo_f = max(lo, vstar)
        xs = np.linspace(lo_f, hi, 9, dtype=np.float64)
        t = xs - lo
        A = np.stack([np.ones_like(t), t, t * t, t ** 3], axis=1)
        coef, *_ = np.linalg.lstsq(A, g(xs), rcond=None)
        return tuple(np.float32(v) for v in coef)

    return fit


def _ratio_octaves():
    return [(e, 3) for e in range(-7, 4)] + [(e, e) for e in range(4, 10)]


def _func_id_of(name):
    d = _stock_dir()
    info = json.load(open(f"{d}/act_info.json"))
    for s in info["act_func_sets"]:
        if name in s["act"]:
            j = json.load(open(f"{d}/{s['profile_json']}"))
            for e in j["profile_meta_data"]:
                nm = e["func_name"]
                if nm == name or nm.rsplit("_", 1)[0] == name or nm.startswith(name + "_"):
                    return e["func_id"]
    raise KeyError(name)


def build_act_root(outdir, curves1024):
    os.makedirs(outdir, exist_ok=True)
    sig_j, sig_c, sig_b = _load_set("sigmoid_and_others")
    sq_j, _, _ = _load_set("sqrt_and_others")
    tmpl = next(dict(e) for e in sq_j["profile_meta_data"] if e["func_name"].startswith("sqrt"))
    info_sets = []

    expsqrt = lambda x: np.exp(-4.0 * np.sqrt(np.asarray(x, np.float64)))
    sigsh = lambda x: 1.0 / (1.0 + np.exp(-(np.asarray(x, np.float64) - 16.0)))
    recip4 = lambda x: 1.0 / np.maximum(np.asarray(x, np.float64), 1e-4)
    ES_OCT = [(e, 1) for e in range(-20, -7)] + [(e, 3) for e in range(-7, -2)] + [(e, 4) for e in range(-2, 2)]
    SIG_OCT = [(1, 2), (2, 3), (3, 5), (4, 6)]
    R4_OCT = [(e, 4) for e in range(-14, -12)] + [(e, 3) for e in range(-12, 0)] + [(0, 1)]

    for k in range(B):
        sb = _SetBuilder(f"cust_tc_{k}")
        fit = _make_ratio_fit(curves1024[k])
        sb.add_pwp_func(TC_NAMES[k], _func_id_of(TC_NAMES[k]), _ratio_octaves(), fit,
                        fzero=1.0, small_const=1.0,
                        large_const=float(curves1024[k][1023]), template_entry=tmpl)
        sb.add_pwp_func("exp", _func_id_of("exp"), ES_OCT,
                        lambda lo, hi: _fit_cubic(expsqrt, lo, hi),
                        fzero=1.0, small_const=float(np.exp(-4.0 * np.sqrt(2.0 ** -20))),
                        large_const=float(np.exp(-8.0)), template_entry=tmpl)
        sb.add_pwp_func("sigmoid", _func_id_of("sigmoid"), SIG_OCT,
                        lambda lo, hi: _fit_cubic(sigsh, lo, hi),
                        fzero=0.0, small_const=float(sigsh(2.0)),
                        large_const=1.0, template_entry=tmpl)
        sb.add_pwp_func(RECIP4_NAME, _func_id_of(RECIP4_NAME), R4_OCT,
                        lambda lo, hi: _fit_cubic(recip4, lo, hi),
                        fzero=1e4, small_const=1e4, large_const=1.0,
                        template_entry=tmpl)
        for f in ("identity", "copy"):
            try:
                sb.copy_stock_func(sig_j, sig_c, sig_b, f, 1)
            except (KeyError, AssertionError):
                pass
        info_sets.append(sb.finalize(outdir))

    json.dump({"pwp_file_keys": ["bkt_bin", "ctrl_bin", "profile_json"],
               "act_func_sets": info_sets}, open(f"{outdir}/act_info.json", "w"))
    return outdir


# ----------------------------------------------------------------------------
# bass kernel construction
# ----------------------------------------------------------------------------

def _split_drain_waits(nc, mybir):
    """This container's walrus supports few sem-waits per instruction (1 on
    Drain/CTRL, ~2-3 on compute).  Spill excess waits onto preceding 1-wait
    Drains on the same engine."""
    for f in nc.m.functions:
        for bb in f.blocks:
            newinsts = []
            for inst in bb.instructions:
                si = inst.sync_info
                keep = 1
                if si is not None and len(si.on_wait) > keep:
                    waits = list(si.on_wait)
                    extra, rest = waits[:-keep], waits[-keep:]
                    for k, w in enumerate(extra):
                        d = mybir.InstDrain(name=f"{inst.name}-ws{k}",
                                            engine=inst.engine, ins=[], outs=[])
                        d.sync_info = mybir.SyncInfo(on_wait=[w], on_update=[])
                        newinsts.append(d)
                    si.on_wait = rest
                newinsts.append(inst)
            bb.instructions = newinsts


def build_kernel(nonce):
    import concourse.bass as bass
    import concourse.mybir as mybir
    from concourse.tile import TileContext

    AF = mybir.ActivationFunctionType
    dt = mybir.dt.float32
    Op = mybir.AluOpType
    AX = mybir.AxisListType

    TC_AF = [AF.from_pwp(n) for n in TC_NAMES]
    ES_AF = [AF.from_pwp(n) for n in ES_NAMES]
    R4_AF = AF.from_pwp(RECIP4_NAME)

    import concourse.bass_isa as bass_isa
    nc = bass.Bass()
    img = nc.dram_tensor(f"img_{nonce}", [C, P, FREE], dt, kind="ExternalInput")
    prm = nc.dram_tensor("prm", [P, NP_], dt, kind="ExternalInput")
    out = nc.dram_tensor("out", [C, P, FREE], dt, kind="ExternalOutput")

    with TileContext(nc) as tc:
        pid = nc.partition_id()
        from contextlib import ExitStack
        with (
            tc.tile_pool(name="planesPU", bufs=1) as planes_pool,
            tc.tile_pool(name="consts", bufs=1) as consts_pool,
        ):
            Pp = planes_pool.tile([P, FREE], dt, tag="Pp")
            Up = planes_pool.tile([P, FREE], dt, tag="Up")
            stkp = ExitStack()
            stk = ExitStack()
            lpool = stk.enter_context(tc.tile_pool(name="lplane", bufs=1))
            ws = stk.enter_context(tc.tile_pool(name="ws1", bufs=9))
            io = stk.enter_context(tc.tile_pool(name="io1", bufs=5))
            tiny = stk.enter_context(tc.tile_pool(name="tiny1", bufs=1))
            Lp = lpool.tile([P, FREE], dt, tag="Lp")
            pr = consts_pool.tile([P, NP_], dt)
            nc.sync.dma_start(pr[:, :], prm[:, :])

            def sc(name):
                i = PRM[name]
                return pr[:, i:i + 1]

            accs = consts_pool.tile([P, NCH], dt, tag="accs")
            sm = []
            for k in range(4):
                smk = consts_pool.tile([P, 1], dt, tag=f"sm{k}", name=f"sm{k}")
                sm.append(smk)

            def col(j):
                return slice(j * F, (j + 1) * F)

            # ---------------- phase 1: stream image -> L plane + sig1 accum
            for j in range(NCH):
                r = io.tile([P, F], dt, tag="i")
                g = io.tile([P, F], dt, tag="i")
                b = io.tile([P, F], dt, tag="i")
                nc.sync.dma_start(r[:, :], img[0, :, col(j)])
                nc.sync.dma_start(g[:, :], img[1, :, col(j)])
                nc.sync.dma_start(b[:, :], img[2, :, col(j)])
                # img1_c = clip(x*A_c + t, 0, hi)  (in place)
                for tch, an in ((r, "A_r"), (g, "A_g"), (b, "A_b")):
                    nc.vector.tensor_scalar(tch[:, :], tch[:, :], sc(an), sc("t"),
                                            Op.mult, Op.add)
                    nc.vector.tensor_scalar(tch[:, :], tch[:, :], 0.0, sc("hi"),
                                            Op.max, Op.min)
                t1 = ws.tile([P, F], dt, tag="w")
                nc.vector.tensor_scalar(t1[:, :], r[:, :], 0.2126, None, Op.mult)
                nc.vector.scalar_tensor_tensor(t1[:, :], g[:, :], 0.7152, t1[:, :],
                                               Op.mult, Op.add)
                nc.vector.scalar_tensor_tensor(Lp[:, col(j)], b[:, :], 0.0722, t1[:, :],
                                               Op.mult, Op.add)
                sg = ws.tile([P, F], dt, tag="w")
                nc.scalar.activation(sg[:, :], Lp[:, col(j)], AF.Sigmoid,
                                     bias=sc("b1"), scale=float(SIG_AFF[0][0]),
                                     accum_out=accs[:, j:j + 1])

            ones = consts_pool.tile([P, 1], dt, tag="ones")
            nc.vector.memset(ones[:, :], 1.0)
            psum = stkp.enter_context(tc.tile_pool(name="psum", bufs=4, space="PSUM"))

            def finish_mean(k):
                # accs cols -> mean -> sm[k] = s_{k+1} * mean  (all [P,1])
                tot = tiny.tile([P, 1], dt, tag="tot", name=f"tot{k}")
                nc.vector.tensor_reduce(tot[:, :], accs[:, :], AX.X, Op.add)
                ps1 = psum.tile([1, 1], dt, tag="ps1", name=f"ps1_{k}")
                nc.tensor.matmul(ps1[:, :], tot[:, :], ones[:, :], start=True, stop=True)
                sb1 = tiny.tile([1, 1], dt, tag="sb1", name=f"sb1_{k}")
                nc.vector.tensor_copy(sb1[:, :], ps1[:, :])
                ps2 = psum.tile([P, 1], dt, tag="ps2", name=f"ps2_{k}")
                nc.tensor.matmul(ps2[:, :], ones[0:1, 0:1].to_broadcast((1, P)),
                                 sb1[:, :], start=True, stop=True)
                nc.vector.tensor_scalar(sm[k][:, :], ps2[:, :], sc("invN"), None,
                                        Op.mult)

            finish_mean(0)

            # ---------------- region chain on L plane
            for k in range(4):
                scale_k, bias_k = SIG_AFF[k]
                sname = f"s{k + 1}"
                for j in range(NCH):
                    Lj = Lp[:, col(j)]
                    sg = ws.tile([P, F], dt, tag="w")
                    nc.scalar.activation(sg[:, :], Lj, AF.Sigmoid,
                                         bias=sc(f"b{k + 1}"), scale=float(scale_k))
                    rec = ws.tile([P, F], dt, tag="w")
                    nc.scalar.activation(rec[:, :], Lj, R4_AF)
                    msk = ws.tile([P, F], dt, tag="w")
                    nc.vector.tensor_scalar(msk[:, :], Lj, 1e-4, None, Op.is_gt)
                    # Lnew = clip(L + s*(sig - m), 0, 1)  (reference rounding order)
                    y = ws.tile([P, F], dt, tag="w")
                    nc.vector.tensor_scalar(y[:, :], sg[:, :], sm[k][:, 0:1], None,
                                            Op.subtract)
                    nc.vector.scalar_tensor_tensor(y[:, :], y[:, :], sc(sname), Lj,
                                                   Op.mult, Op.add)
                    nc.vector.tensor_scalar(Lj, y[:, :], 0.0, 1.0, Op.max, Op.min)
                    # r = 1 + mask*(Lnew*rec - 1)
                    q = ws.tile([P, F], dt, tag="w")
                    nc.vector.tensor_tensor(q[:, :], Lj, rec[:, :], Op.mult)
                    nc.vector.scalar_tensor_tensor(q[:, :], q[:, :], 1.0, msk[:, :],
                                                   Op.subtract, Op.mult)
                    if k == 0:
                        nc.vector.tensor_scalar(Pp[:, col(j)], q[:, :], 1.0, None, Op.add)
                    else:
                        nc.vector.tensor_scalar(q[:, :], q[:, :], 1.0, None, Op.add)
                        nc.gpsimd.tensor_tensor(Pp[:, col(j)], Pp[:, col(j)], q[:, :],
                                                Op.mult)
                        if k == 1:
                            nc.vector.tensor_scalar(Up[:, col(j)], q[:, :], 1.0, None,
                                                    Op.min)
                        else:
                            nc.gpsimd.tensor_tensor(q[:, :], q[:, :], Up[:, col(j)],
                                                    Op.mult)
                            nc.vector.tensor_scalar(Up[:, col(j)], q[:, :], 1.0, None,
                                                    Op.min)
                    if k < 3:
                        sg2 = ws.tile([P, F], dt, tag="w")
                        nc.scalar.activation(sg2[:, :], Lj, AF.Sigmoid,
                                             bias=sc(f"b{k + 2}"),
                                             scale=float(SIG_AFF[k + 1][0]),
                                             accum_out=accs[:, j:j + 1])
                if k < 3:
                    finish_mean(k + 1)

            # ---------------- final pass (per-core branch: custom ACT funcs)
            stk.close()
            stk2 = ExitStack()
            ws = stk2.enter_context(tc.tile_pool(name="ws2", bufs=14))
            io = stk2.enter_context(tc.tile_pool(name="io2", bufs=6))
            for core in range(B):
                with tc.If(pid == core):
                    for j in range(NCH):
                        r = io.tile([P, F], dt, tag="i")
                        g = io.tile([P, F], dt, tag="i")
                        b = io.tile([P, F], dt, tag="i")
                        nc.sync.dma_start(r[:, :], img[0, :, col(j)])
                        nc.sync.dma_start(g[:, :], img[1, :, col(j)])
                        nc.sync.dma_start(b[:, :], img[2, :, col(j)])
                        chans = []
                        for tch, an in ((r, "A_r"), (g, "A_g"), (b, "A_b")):
                            nc.vector.tensor_scalar(tch[:, :], tch[:, :], sc(an),
                                                    sc("t"), Op.mult, Op.add)
                            nc.vector.tensor_scalar(tch[:, :], tch[:, :], 0.0,
                                                    sc("hi"), Op.max, Op.min)
                            x5 = ws.tile([P, F], dt, tag="w")
                            nc.gpsimd.tensor_tensor(x5[:, :], tch[:, :], Pp[:, col(j)],
                                                    Op.mult)
                            nc.vector.tensor_tensor(x5[:, :], x5[:, :], Up[:, col(j)],
                                                    Op.min)
                            chans.append(x5)
                        x5r, x5g, x5b = chans
                        L5 = ws.tile([P, F], dt, tag="w")
                        nc.vector.tensor_scalar(L5[:, :], x5r[:, :], 0.2126, None, Op.mult)
                        nc.vector.scalar_tensor_tensor(L5[:, :], x5g[:, :], 0.7152,
                                                       L5[:, :], Op.mult, Op.add)
                        nc.vector.scalar_tensor_tensor(L5[:, :], x5b[:, :], 0.0722,
                                                       L5[:, :], Op.mult, Op.add)
                        tr = ws.tile([P, F], dt, tag="w")
                        nc.scalar.activation(tr[:, :], L5[:, :], TC_AF[core],
                                             scale=1023.0)
                        msk = ws.tile([P, F], dt, tag="w")
                        nc.vector.tensor_scalar(msk[:, :], L5[:, :], 1e-5, None, Op.is_gt)
                        nc.vector.scalar_tensor_tensor(tr[:, :], tr[:, :], 1.0,
                                                       msk[:, :], Op.subtract, Op.mult)
                        nc.vector.tensor_scalar(tr[:, :], tr[:, :], 1.0, None, Op.add)
                        # img_tc = min(x5*tr, 1); in place into x5
                        for x5 in chans:
                            nc.vector.tensor_tensor(x5[:, :], x5[:, :], tr[:, :], Op.mult)
                            nc.vector.tensor_scalar(x5[:, :], x5[:, :], 1.0, None, Op.min)
                        # vibrance
                        Lv = ws.tile([P, F], dt, tag="w")
                        nc.vector.tensor_scalar(Lv[:, :], x5r[:, :], 0.2126, None, Op.mult)
                        nc.vector.scalar_tensor_tensor(Lv[:, :], x5g[:, :], 0.7152,
                                                       Lv[:, :], Op.mult, Op.add)
                        nc.vector.scalar_tensor_tensor(Lv[:, :], x5b[:, :], 0.0722,
                                                       Lv[:, :], Op.mult, Op.add)
                        chs = []
                        ss = ws.tile([P, F], dt, tag="w")
                        s2 = ws.tile([P, F], dt, tag="w")
                        for i, x5 in enumerate(chans):
                            ch = ws.tile([P, F], dt, tag="w")
                            nc.gpsimd.tensor_tensor(ch[:, :], x5[:, :], Lv[:, :],
                                                    Op.subtract)
                            chs.append(ch)
                        nc.vector.tensor_tensor(ss[:, :], chs[0][:, :], chs[0][:, :],
                                                Op.mult)
                        nc.vector.tensor_tensor(s2[:, :], chs[1][:, :], chs[1][:, :],
                                                Op.mult)
                        nc.vector.tensor_tensor(ss[:, :], ss[:, :], s2[:, :], Op.add)
                        nc.vector.tensor_tensor(s2[:, :], chs[2][:, :], chs[2][:, :],
                                                Op.mult)
                        nc.vector.tensor_tensor(ss[:, :], ss[:, :], s2[:, :], Op.add)
                        gn = ws.tile([P, F], dt, tag="w")
                        nc.scalar.activation(gn[:, :], ss[:, :], ES_AF[core], bias=sc("esb"))
                        nc.vector.tensor_scalar(gn[:, :], gn[:, :], sc("v"), 1.0,
                                                Op.mult, Op.add)
                        nc.vector.tensor_scalar(gn[:, :], gn[:, :], 4.0, 0.2,
                                                Op.min, Op.max)
                        # out_v = clip(Lv + ch*gn, 0, 1), then saturation:
                        # out = clip(out_v*gs + Ls*(1-gs), 0, 1)
                        for i, ch in enumerate(chs):
                            nc.vector.tensor_tensor(ch[:, :], ch[:, :], gn[:, :], Op.mult)
                            nc.vector.tensor_tensor(ch[:, :], ch[:, :], Lv[:, :], Op.add)
                            nc.vector.tensor_scalar(ch[:, :], ch[:, :], 0.0, 1.0,
                                                    Op.max, Op.min)
                        Ls = ws.tile([P, F], dt, tag="w")
                        nc.vector.tensor_scalar(Ls[:, :], chs[0][:, :], 0.2126, None,
                                                Op.mult)
                        nc.vector.scalar_tensor_tensor(Ls[:, :], chs[1][:, :], 0.7152,
                                                       Ls[:, :], Op.mult, Op.add)
                        nc.vector.scalar_tensor_tensor(Ls[:, :], chs[2][:, :], 0.0722,
                                                       Ls[:, :], Op.mult, Op.add)
                        Bs = ws.tile([P, F], dt, tag="w")
                        nc.vector.tensor_scalar(Bs[:, :], Ls[:, :], sc("omgs"), None,
                                                Op.mult)
                        for i, ch in enumerate(chs):
                            oc = io.tile([P, F], dt, tag="o")
                            nc.vector.scalar_tensor_tensor(oc[:, :], ch[:, :], sc("gs"),
                                                           Bs[:, :], Op.mult, Op.add)
                            nc.vector.tensor_scalar(oc[:, :], oc[:, :], 0.0, 1.0,
                                                    Op.max, Op.min)
                            nc.sync.dma_start(out[i, :, col(j)], oc[:, :])
            stk2.close()
            stkp.close()

    _split_drain_waits(nc, mybir)
    return nc


# ----------------------------------------------------------------------------
# host side
# ----------------------------------------------------------------------------

def _host_params(inputs):
    def denorm(lo, hi, v):
        return lo + 0.5 * (v + 1.0) * (hi - lo)

    t64 = np.float64
    temp = denorm(2000.0, 50000.0, inputs["temperature_n"].astype(t64))
    tint = denorm(-150.0, 150.0, inputs["tint_n"].astype(t64))
    expo = denorm(-5.0, 5.0, inputs["exposure_n"].astype(t64))
    contr = denorm(-100.0, 100.0, inputs["contrast_n"].astype(t64))
    hl = denorm(-100.0, 100.0, inputs["highlights_n"].astype(t64))
    sh = denorm(-100.0, 100.0, inputs["shadows_n"].astype(t64))
    wh = denorm(-100.0, 100.0, inputs["whites_n"].astype(t64))
    bl = denorm(-100.0, 100.0, inputs["blacks_n"].astype(t64))
    vib = denorm(-100.0, 100.0, inputs["vibrance_n"].astype(t64))
    sat = denorm(-100.0, 100.0, inputs["saturation_n"].astype(t64))

    tr = 6500.0 / np.clip(temp, 2000.0, 50000.0)
    red = np.sqrt(tr)
    blue = 1.0 / np.sqrt(tr)
    ts = np.clip(tint / 150.0, -1.5, 1.5)
    green = 1.0 - 0.1 * ts
    red = red * (1.0 + 0.05 * ts)
    blue = blue * (1.0 - 0.05 * ts)
    gains = np.stack([red, green, blue], axis=1)  # [B,3]
    norm = np.maximum(gains.max(axis=1), 1e-4)
    G = gains / norm[:, None]
    e = np.power(2.0, expo)
    f = 1.0 + contr / 100.0
    A = G * (e * f)[:, None]
    t = 0.5 - 0.5 * f
    u = np.minimum(4.0 * e, 4.0)
    hi = np.clip(u * f + t, 0.0, 1.0)

    prm = np.zeros((B, NP_), np.float64)
    prm[:, PRM["A_r"]] = A[:, 0]
    prm[:, PRM["A_g"]] = A[:, 1]
    prm[:, PRM["A_b"]] = A[:, 2]
    prm[:, PRM["t"]] = t
    prm[:, PRM["hi"]] = hi
    prm[:, PRM["s1"]] = hl / 100.0
    prm[:, PRM["s2"]] = sh / 100.0
    prm[:, PRM["s3"]] = wh / 100.0
    prm[:, PRM["s4"]] = bl / 100.0
    prm[:, PRM["v"]] = vib / 100.0
    prm[:, PRM["gs"]] = 1.0 + sat / 100.0
    prm[:, PRM["omgs"]] = -sat / 100.0
    prm[:, PRM["invN"]] = 1.0 / NPIX
    for k in range(4):
        prm[:, PRM[f"b{k + 1}"]] = SIG_AFF[k][1] + 16.0
    prm[:, PRM["esb"]] = 1e-6
    return prm.astype(np.float32)


def _curves1024(tone_curve):
    c = tone_curve.astype(np.float64)  # [B,256]
    src = np.arange(1024) * (255.0 / 1023.0)
    i0 = np.floor(src).astype(int)
    i1 = np.minimum(i0 + 1, 255)
    w = src - i0
    return c[:, i0] * (1 - w) + c[:, i1] * w


_CACHE = {}
LAST_EXEC_NS = None
PROFILE = False


def kernel(**inputs):
    img = np.ascontiguousarray(inputs["image"], dtype=np.float32)
    curves = _curves1024(np.asarray(inputs["tone_curve"], np.float32))
    prm = _host_params({k: np.asarray(v, np.float32) for k, v in inputs.items()
                        if k != "image"})

    key = hashlib.sha256(curves.tobytes()).hexdigest()[:12]
    workdir = os.path.join(tempfile.gettempdir(), f"editlayer_{key}")
    actroot = os.path.join(workdir, "actroot")
    if key not in _CACHE:
        os.makedirs(workdir, exist_ok=True)
        build_act_root(actroot, curves)
        os.environ["BASS_ACT_ROOT_JSON_PATH"] = os.path.join(actroot, "act_info.json")
        nc = build_kernel(key)
        _CACHE[key] = nc
    nc = _CACHE[key]
    os.environ["BASS_ACT_ROOT_JSON_PATH"] = os.path.join(actroot, "act_info.json")

    from concourse.bass_utils import run_bass_kernel_spmd
    global LAST_EXEC_NS
    in_maps = []
    for k in range(B):
        in_maps.append({
            f"img_{key}": img[k].reshape(C, P, FREE),
            "prm": np.broadcast_to(prm[k], (P, NP_)).copy(),
        })
    want_trace = bool(globals().get("PROFILE", False))
    try:
        res = run_bass_kernel_spmd(nc, in_maps, core_ids=list(range(B)),
                                   trace=want_trace)
    except Exception:
        if not want_trace:
            raise
        res = run_bass_kernel_spmd(nc, in_maps, core_ids=list(range(B)))
    if getattr(res, "exec_time_ns", None):
        LAST_EXEC_NS = res.exec_time_ns
    outs = [res.results[k]["out"].reshape(C, H, W) for k in range(B)]
    return np.stack(outs, axis=0)


if __name__ == "__main__":
    import reference
    inputs = {k: np.asarray(v) for k, v in reference.setup_inputs().items()}
    outp = kernel(**inputs)
    exp = np.asarray(reference.reference(**inputs))
    err = np.abs(outp - exp)
    denom = np.abs(exp).max()
    print("max abs err:", err.max(), "rel:", err.max() / denom)



# revision 6
# speedup vs baseline: 13894.6632x; 13894.6632x over previous
"""Trainium2 Bass kernel for nn_DifferentiableEditLayer (v2, optimized).

Structure (per core = one batch sample, data parallel across 8 cores):
  Phase A: stream image (read 1), stage-1 (WB/exposure/contrast folded into
    per-channel affine+clip), luma plane L1 (f32), sigmoid-1 plane + mean
    accumulation in one ACTIVATE.
  Region chain: 4 sweeps over the f32 luma plane. Each sweep reads the
    cached sigmoid plane (written by the previous sweep's mean-accum
    ACTIVATE), computes Lnew via DVE affine + exact ACT min1 table,
    r-products into bf16 P/S planes with fused STT updates.
  Final: b1 (read 2) recompute stage-1, x5 = min(x1*P, S), L5 bf16 plane;
    b2: per-core tone-curve ratio via custom ACT tables under 8 If blocks;
    b3 (read 3) recompute x5, apply tone ratio, vibrance, saturation, write.
  All cancellation-critical luma math stays f32; multiplicative channel math
  runs bf16. Work is split across DVE / GpSimd / ACT engines.
"""
import os, json, struct, hashlib, tempfile
import numpy as np

# ----------------------------------------------------------------------------
# constants
# ----------------------------------------------------------------------------
B, C, H, W = 8, 3, 1024, 1536
NPIX = H * W            # 1,572,864
P = 128                 # SBUF partitions
FREE = NPIX // P        # 12288
F = 1024                # chunk free size
NCH = FREE // F         # 12 chunks

TC_NAMES = ["sin", "arctan", "erf", "gelu", "silu", "derivative_silu",
            "gelu_apprx_tanh", "derivative_gelu"]
ES_NAME = "exp"         # expsqrt hijacks 'exp'
RECIP4_NAME = "ln"      # 1/max(x,1e-4) hijacks 'ln'
MIN1_NAME = "tanh"      # min(x,1) hijacks 'tanh'
SIG_NAME = "sigmoid"    # shifted sigmoid hijacks 'sigmoid'

# region (pivot, width) and derived sigmoid affine (scale, bias), compile-time
REGIONS = [(0.7, 0.1), (0.3, 0.12), (0.9, 0.08), (0.1, 0.08)]
SIG_AFF = [(1.0 / w, -p / w) for (p, w) in REGIONS]

# prm layout
PRM = dict(A_r=0, A_g=1, A_b=2, t=3, hi=4, s1=5, s2=6, s3=7, s4=8,
           v=9, gs=10, omgs=11, invN=12, b1=13, b2=14, b3=15, b4=16, esb=17)
NP_ = 18


# ----------------------------------------------------------------------------
# custom ACT table generation
# ----------------------------------------------------------------------------

def _stock_dir():
    import neuronxcc
    return os.path.join(os.path.dirname(neuronxcc.__file__), "pwp", "pwp_bin_trainium")


def _load_set(name):
    d = _stock_dir()
    j = json.load(open(f"{d}/{name}.json"))
    ctrl = open(f"{d}/{name}_ctrl.bin", "rb").read()
    bkt = open(f"{d}/{name}_bkt.bin", "rb").read()
    return j, ctrl, bkt


def _func_span(j, fname, kind):
    key = "func_to_bkt_start_idx" if kind == "b" else "func_to_ctl_start_idx"
    cnt = j["bkt_entry_cnt"] if kind == "b" else j["ctl_entry_cnt"]
    starts = j[key]
    s = starts[fname]
    nxt = [v for v in starts.values() if v > s]
    return s, (min(nxt) if nxt else cnt)


class _SetBuilder:
    def __init__(self, name):
        self.name = name
        self.ctl, self.bkt, self.profile = [], [], []
        self.f2b, self.f2c, self.fe2b, self.fe2c, self.act = {}, {}, {}, {}, {}

    def copy_stock_func(self, set_json, ctrl_bin, bkt_bin, fname, ulp):
        b0, b1 = _func_span(set_json, fname, "b")
        c0, c1 = _func_span(set_json, fname, "c")
        boff = len(self.bkt) - b0
        coff = len(self.ctl) - c0
        for i in range(b0, b1):
            self.bkt.append(struct.unpack_from("<5f", bkt_bin, i * 32))
        for i in range(c0, c1):
            d = struct.unpack_from("<I", ctrl_bin, i * 32)[0]
            self.ctl.append((d & ~0x7FF) | (((d & 0x7FF) + boff) & 0x7FF))
        ent = None
        for e in set_json["profile_meta_data"]:
            nm = e["func_name"]
            if nm == fname or nm.rsplit("_", 1)[0] == fname or nm.startswith(fname + "_"):
                ent = dict(e)
                break
        assert ent is not None, f"no profile entry for {fname}"
        for k in ("pwl_control_base_pos", "pwl_control_base_neg"):
            ent[k] = ent.get(k, 0) + coff
        for k in ("pos_small_signal_pwl_control", "neg_small_signal_pwl_control",
                  "pos_large_signal_pwl_control", "neg_large_signal_pwl_control"):
            ent[k] = ent.get(k, 0) + boff
        self.profile.append(ent)
        self.f2b[fname] = b0 + boff
        self.f2c[fname] = c0 + coff
        self.fe2b[fname] = {k: [v + boff for v in vs] for k, vs in set_json["func_exp_to_bkt_start_idx"].get(fname, {}).items()}
        self.fe2c[fname] = {k: [v + coff for v in vs] for k, vs in set_json["func_exp_to_ctl_start_idx"].get(fname, {}).items()}
        self.act[fname] = ulp

    def add_pwp_func(self, fname, func_id, octaves, fit_fn, fzero, small_const,
                     large_const, template_entry, ulp=4):
        bstart, cstart = len(self.bkt), len(self.ctl)
        fe2b, fe2c = {}, {}
        for (e, nb) in octaves:
            n = 1 << nb
            lo_oct = float(2.0 ** e)
            w = lo_oct / n
            fe2c[str(e)] = [len(self.ctl)]
            fe2b[str(e)] = [len(self.bkt)]
            self.ctl.append((len(self.bkt) & 0x7FF) | ((23 - nb) << 11) | (nb << 16))
            for i in range(n):
                lo = lo_oct + i * w
                d0, d1, d2, d3 = fit_fn(lo, lo + w)
                self.bkt.append((d0, d1, d2, d3, np.float32(lo)))
        small_bkt = len(self.bkt)
        self.bkt.append((small_const, 0.0, 0.0, 0.0, 0.0))
        large_bkt = len(self.bkt)
        self.bkt.append((large_const, 0.0, 0.0, 0.0, 0.0))
        e_lo, e_hi = octaves[0][0], octaves[-1][0]
        ent = dict(template_entry)
        ent.update(func_name=fname + "_4p", func_id=func_id, symmetry_point=0,
                   sym_invert_sign_point=0, symmetry_opt_en=0,
                   symmetry_opt_use_neg_region=0, imm_bias=0, exp_offset=e_lo,
                   pwl_control_base_pos=cstart, pwl_control_base_neg=cstart,
                   small_pos_signal_exp_threshold=e_lo + 127,
                   pos_small_signal_pwl_control=small_bkt,
                   small_neg_signal_exp_threshold=0,
                   neg_small_signal_pwl_control=small_bkt,
                   large_pos_signal_exp_threshold=e_hi + 1 + 127,
                   large_pos_signal_mantissa_threshold=0,
                   pos_large_signal_pwl_control=large_bkt,
                   large_neg_signal_exp_threshold=0,
                   large_neg_signal_mantissa_threshold=0,
                   neg_large_signal_pwl_control=small_bkt,
                   fzero_result=int(np.float32(fzero).view(np.uint32)),
                   fnan_result=int(np.float32(fzero).view(np.uint32)),
                   fpinf_result=int(np.float32(large_const).view(np.uint32)),
                   fninf_result=int(np.float32(small_const).view(np.uint32)),
                   fma_const_0=0, fma_const_1=0, fma_indirection_src_sel=0,
                   use_multipass=False,
                   lower_bound=int(np.float32(2.0 ** e_lo).view(np.uint32)),
                   upper_bound=int(np.float32(2.0 ** (e_hi + 1)).view(np.uint32)))
        self.profile.append(ent)
        self.f2b[fname], self.f2c[fname] = bstart, cstart
        self.fe2b[fname], self.fe2c[fname] = fe2b, fe2c
        self.act[fname] = ulp

    def finalize(self, outdir):
        assert len(self.bkt) <= 1536, f"{self.name}: {len(self.bkt)} buckets"
        j = {"bkt_bin": f"{self.name}_bkt.bin", "ctl_bin": f"{self.name}_ctrl.bin",
             "profile_meta_data": self.profile,
             "bkt_entry_cnt": len(self.bkt), "ctl_entry_cnt": len(self.ctl),
             "func_to_bkt_start_idx": self.f2b, "func_to_ctl_start_idx": self.f2c,
             "func_exp_to_bkt_start_idx": self.fe2b,
             "func_exp_to_ctl_start_idx": self.fe2c}
        json.dump(j, open(f"{outdir}/{self.name}.json", "w"))
        with open(f"{outdir}/{self.name}_ctrl.bin", "wb") as f:
            for d in self.ctl:
                f.write(struct.pack("<I", d) + b"\0" * 28)
        with open(f"{outdir}/{self.name}_bkt.bin", "wb") as f:
            for b in self.bkt:
                f.write(struct.pack("<5f", *b) + b"\0" * 12)
        return {"name": self.name, "bkt_bin": j["bkt_bin"], "ctrl_bin": j["ctl_bin"],
                "profile_json": f"{self.name}.json", "act": self.act}


def _fit_cubic(fn, lo, hi, M=9):
    xs = np.linspace(lo, hi, M, dtype=np.float64)
    t = xs - lo
    A = np.stack([np.ones_like(t), t, t * t, t ** 3], axis=1)
    c, *_ = np.linalg.lstsq(A, fn(xs), rcond=None)
    return tuple(np.float32(v) for v in c)


def _make_ratio_fit(curve1024):
    c = np.asarray(curve1024, np.float64)
    vstar = 1023.0e-5

    def g(v):
        v = np.asarray(v, np.float64)
        i = np.clip(np.floor(v).astype(int), 0, 1022)
        w = v - i
        tgt = c[i] * (1 - w) + c[i + 1] * w
        tgt = np.where(v >= 1023, c[1023], tgt)
        return tgt * 1023.0 / np.maximum(v, vstar)

    def fit(lo, hi):
        if hi <= vstar:
            return (np.float32(1.0), np.float32(0), np.float32(0), np.float32(0))
        lo_f = max(lo, vstar)
        xs = np.linspace(lo_f, hi, 9, dtype=np.float64)
        t = xs - lo
        A = np.stack([np.ones_like(t), t, t * t, t ** 3], axis=1)
        coef, *_ = np.linalg.lstsq(A, g(xs), rcond=None)
        return tuple(np.float32(v) for v in coef)

    return fit


# octave layouts (v2): tc 594, sigmoid 410, recip4 180, min1 34, expsqrt 134
def _ratio_octaves():
    return ([(e, 3) for e in range(-7, 4)]
            + [(4, 4), (5, 5), (6, 6), (7, 7), (8, 7), (9, 8)])


SIG_OCT = [(1, 3), (2, 4), (3, 7), (4, 8)]
R4_OCT = [(-14, 6)] + [(e, 3) for e in range(-13, 0)] + [(0, 1)]
ES_OCT = [(e, 1) for e in range(-20, -7)] + [(e, 3) for e in range(-7, -2)] + [(e, 4) for e in range(-2, 2)]
MIN1_OCT = [(e, 0) for e in range(-30, 2)]


def _fit_min1(lo, hi):
    # exact: identity below 1, constant 1 at/above 1 (octave boundaries align)
    if lo >= 1.0:
        return (np.float32(1.0), np.float32(0.0), np.float32(0.0), np.float32(0.0))
    return (np.float32(lo), np.float32(1.0), np.float32(0.0), np.float32(0.0))


def _func_id_of(name):
    d = _stock_dir()
    info = json.load(open(f"{d}/act_info.json"))
    for s in info["act_func_sets"]:
        if name in s["act"]:
            j = json.load(open(f"{d}/{s['profile_json']}"))
            for e in j["profile_meta_data"]:
                nm = e["func_name"]
                if nm == name or nm.rsplit("_", 1)[0] == name or nm.startswith(name + "_"):
                    return e["func_id"]
    raise KeyError(name)


def build_act_root(outdir, curves1024):
    os.makedirs(outdir, exist_ok=True)
    sig_j, sig_c, sig_b = _load_set("sigmoid_and_others")
    sq_j, _, _ = _load_set("sqrt_and_others")
    tmpl = next(dict(e) for e in sq_j["profile_meta_data"] if e["func_name"].startswith("sqrt"))
    info_sets = []

    expsqrt = lambda x: np.exp(-4.0 * np.sqrt(np.asarray(x, np.float64)))
    sigsh = lambda x: 1.0 / (1.0 + np.exp(-(np.asarray(x, np.float64) - 16.0)))
    recip4 = lambda x: 1.0 / np.maximum(np.asarray(x, np.float64), 1e-4)

    for k in range(B):
        sb = _SetBuilder(f"cust_tc_{k}")
        fit = _make_ratio_fit(curves1024[k])
        sb.add_pwp_func(TC_NAMES[k], _func_id_of(TC_NAMES[k]), _ratio_octaves(), fit,
                        fzero=1.0, small_const=1.0,
                        large_const=float(curves1024[k][1023]), template_entry=tmpl)
        sb.add_pwp_func(ES_NAME, _func_id_of(ES_NAME), ES_OCT,
                        lambda lo, hi: _fit_cubic(expsqrt, lo, hi),
                        fzero=1.0, small_const=float(np.exp(-4.0 * np.sqrt(2.0 ** -20))),
                        large_const=float(np.exp(-8.0)), template_entry=tmpl)
        sb.add_pwp_func(SIG_NAME, _func_id_of(SIG_NAME), SIG_OCT,
                        lambda lo, hi: _fit_cubic(sigsh, lo, hi),
                        fzero=0.0, small_const=float(sigsh(2.0)),
                        large_const=1.0, template_entry=tmpl)
        sb.add_pwp_func(RECIP4_NAME, _func_id_of(RECIP4_NAME), R4_OCT,
                        lambda lo, hi: _fit_cubic(recip4, lo, hi),
                        fzero=1e4, small_const=1e4, large_const=1.0,
                        template_entry=tmpl)
        sb.add_pwp_func(MIN1_NAME, _func_id_of(MIN1_NAME), MIN1_OCT,
                        _fit_min1, fzero=0.0, small_const=0.0,
                        large_const=1.0, template_entry=tmpl, ulp=1)
        for f in ("identity", "copy"):
            try:
                sb.copy_stock_func(sig_j, sig_c, sig_b, f, 1)
            except (KeyError, AssertionError):
                pass
        info_sets.append(sb.finalize(outdir))

    json.dump({"pwp_file_keys": ["bkt_bin", "ctrl_bin", "profile_json"],
               "act_func_sets": info_sets}, open(f"{outdir}/act_info.json", "w"))
    return outdir


# ----------------------------------------------------------------------------
# bass kernel construction
# ----------------------------------------------------------------------------

def _split_drain_waits(nc, mybir):
    """This container's walrus supports few sem-waits per instruction (1 on
    Drain/CTRL, ~2-3 on compute).  Spill excess waits onto preceding 1-wait
    Drains on the same engine."""
    for f in nc.m.functions:
        for bb in f.blocks:
            newinsts = []
            for inst in bb.instructions:
                si = inst.sync_info
                keep = 1
                if si is not None and len(si.on_wait) > keep:
                    waits = list(si.on_wait)
                    extra, rest = waits[:-keep], waits[-keep:]
                    for k, w in enumerate(extra):
                        d = mybir.InstDrain(name=f"{inst.name}-ws{k}",
                                            engine=inst.engine, ins=[], outs=[])
                        d.sync_info = mybir.SyncInfo(on_wait=[w], on_update=[])
                        newinsts.append(d)
                    si.on_wait = rest
                newinsts.append(inst)
            bb.instructions = newinsts


def build_kernel(nonce, repeat=1):
    import concourse.bass as bass
    import concourse.mybir as mybir
    from concourse.tile import TileContext
    from contextlib import ExitStack

    AF = mybir.ActivationFunctionType
    f32 = mybir.dt.float32
    b16 = mybir.dt.float16  # fp16: 10-bit mantissa, same 16-bit DVE perf modes
    Op = mybir.AluOpType
    AX = mybir.AxisListType

    TC_AF = [AF.from_pwp(n) for n in TC_NAMES]
    ES_AF = AF.from_pwp(ES_NAME)
    R4_AF = AF.from_pwp(RECIP4_NAME)
    M1_AF = AF.from_pwp(MIN1_NAME)
    SG_AF = AF.from_pwp(SIG_NAME)

    nc = bass.Bass()
    img = nc.dram_tensor(f"img_{nonce}", [C, P, FREE], f32, kind="ExternalInput")
    prm = nc.dram_tensor("prm", [P, NP_], f32, kind="ExternalInput")
    out = nc.dram_tensor("out", [C, P, FREE], f32, kind="ExternalOutput")

    def col(j):
        return slice(j * F, (j + 1) * F)

    with TileContext(nc) as tc:
        pid = nc.partition_id()
        with (
            tc.tile_pool(name="planes", bufs=1) as planes_pool,
            tc.tile_pool(name="consts", bufs=1) as consts_pool,
        ):
            pr = consts_pool.tile([P, NP_], f32)
            nc.sync.dma_start(pr[:, :], prm[:, :])

            def sc(name):
                i = PRM[name]
                return pr[:, i:i + 1]

            ones = consts_pool.tile([P, 1], f32, tag="ones")
            nc.vector.memset(ones[:, :], 1.0)

            for rep in range(repeat):
                stk = ExitStack()
                chpool = stk.enter_context(
                    tc.tile_pool(name=f"chp{rep}", bufs=1))
                Lp = chpool.tile([P, FREE], f32, tag="Lp")
                SGp = chpool.tile([P, FREE], f32, tag="SGp")
                accs = consts_pool.tile([P, NCH], f32, tag="accs",
                                        name=f"accs{rep}")
                sm = [consts_pool.tile([P, 1], f32, tag=f"sm{k}",
                                       name=f"sm{k}_{rep}") for k in range(4)]
                cs = [consts_pool.tile([P, 1], f32, tag=f"cs{k}",
                                       name=f"cs{k}_{rep}") for k in range(4)]

                # ---------------- phase A: stream -> L plane, sg1 plane+accum
                stkA = ExitStack()
                ioA = stkA.enter_context(tc.tile_pool(name=f"ioA{rep}", bufs=6))
                for j in range(NCH):
                    r = ioA.tile([P, F], f32, tag="i")
                    g = ioA.tile([P, F], f32, tag="i")
                    b = ioA.tile([P, F], f32, tag="i")
                    nc.sync.dma_start(r[:, :], img[0, :, col(j)])
                    nc.sync.dma_start(g[:, :], img[1, :, col(j)])
                    nc.sync.dma_start(b[:, :], img[2, :, col(j)])
                    # stage1 in place; r on DVE, g/b on Pool
                    nc.vector.tensor_scalar(r[:, :], r[:, :], sc("A_r"), sc("t"),
                                            Op.mult, Op.add)
                    nc.vector.tensor_scalar(r[:, :], r[:, :], 0.0, sc("hi"),
                                            Op.max, Op.min)
                    for tch, an in ((g, "A_g"), (b, "A_b")):
                        nc.vector.tensor_scalar(tch[:, :], tch[:, :], sc(an),
                                                sc("t"), Op.mult, Op.add)
                        nc.vector.tensor_scalar(tch[:, :], tch[:, :], 0.0,
                                                sc("hi"), Op.max, Op.min)
                    Lj = Lp[:, col(j)]
                    nc.vector.tensor_scalar(Lj, r[:, :], 0.2126, None, Op.mult)
                    nc.vector.scalar_tensor_tensor(Lj, g[:, :], 0.7152, Lj,
                                                   Op.mult, Op.add)
                    nc.vector.scalar_tensor_tensor(Lj, b[:, :], 0.0722, Lj,
                                                   Op.mult, Op.add)
                    # sg1 plane + mean accumulation in one ACTIVATE
                    nc.scalar.activation(SGp[:, col(j)], Lj, SG_AF,
                                         bias=sc("b1"), scale=float(SIG_AFF[0][0]),
                                         accum_out=accs[:, j:j + 1])
                stkA.close()

                # ---------------- mean finishing (PSUM partition reduce)
                stkp = ExitStack()
                psum = stkp.enter_context(tc.tile_pool(name=f"ps{rep}", bufs=4,
                                                       space="PSUM"))
                tiny = stkp.enter_context(tc.tile_pool(name=f"tiny{rep}", bufs=1))

                def finish_mean(k):
                    tot = tiny.tile([P, 1], f32, tag="tot", name=f"tot{k}_{rep}")
                    nc.vector.tensor_reduce(tot[:, :], accs[:, :], AX.X, Op.add)
                    ps1 = psum.tile([1, 1], f32, tag="ps1", name=f"ps1_{k}_{rep}")
                    nc.tensor.matmul(ps1[:, :], tot[:, :], ones[:, :],
                                     start=True, stop=True)
                    sb1 = tiny.tile([1, 1], f32, tag="sb1", name=f"sb1_{k}_{rep}")
                    nc.vector.tensor_copy(sb1[:, :], ps1[:, :])
                    ps2 = psum.tile([P, 1], f32, tag="ps2", name=f"ps2_{k}_{rep}")
                    nc.tensor.matmul(ps2[:, :], ones[0:1, 0:1].to_broadcast((1, P)),
                                     sb1[:, :], start=True, stop=True)
                    nc.vector.tensor_scalar(sm[k][:, :], ps2[:, :], sc("invN"),
                                            None, Op.mult)
                    # cs[k] = s_k * m_k  (bias for the Lnew clip)
                    nc.vector.tensor_scalar(cs[k][:, :], sm[k][:, :],
                                            sc(f"s{k + 1}"), None, Op.mult)

                finish_mean(0)

                # ---------------- region chain: 4 sweeps over L plane
                stk2 = ExitStack()
                ws = stk2.enter_context(tc.tile_pool(name=f"wsR{rep}", bufs=7))
                Pb = planes_pool.tile([P, FREE], b16, tag="Pb",
                                      name=f"Pb{rep}")
                Sb = planes_pool.tile([P, FREE], b16, tag="Sb",
                                      name=f"Sb{rep}")
                for k in range(4):
                    sname = f"s{k + 1}"
                    for j in range(NCH):
                        Lj = Lp[:, col(j)]
                        SGj = SGp[:, col(j)]
                        rec = ws.tile([P, F], f32, tag="w", name=f"rec{k}_{j}_{rep}")
                        nc.scalar.activation(rec[:, :], Lj, R4_AF)
                        msk = ws.tile([P, F], f32, tag="w", name=f"msk{k}_{j}_{rep}")
                        nc.vector.tensor_scalar(msk[:, :], Lj, 1e-4, None, Op.is_gt)
                        # y = s*sg + L ; y2 = max(y - s*m, 0); Lnew = min1(y2)
                        y = ws.tile([P, F], f32, tag="w", name=f"y{k}_{j}_{rep}")
                        nc.vector.scalar_tensor_tensor(y[:, :], SGj, sc(sname),
                                                       Lj, Op.mult, Op.add)
                        nc.vector.tensor_scalar(y[:, :], y[:, :], cs[k][:, 0:1],
                                                0.0, Op.subtract, Op.max)
                        nc.scalar.activation(Lj, y[:, :], M1_AF)
                        # q = Lnew*rec; t = (q-1)*msk  (t = r - 1)
                        nc.gpsimd.tensor_tensor(rec[:, :], Lj, rec[:, :], Op.mult)
                        th = ws.tile([P, F], f32, tag="w", name=f"th{k}_{j}_{rep}")
                        nc.vector.scalar_tensor_tensor(th[:, :], rec[:, :], 1.0,
                                                       msk[:, :], Op.subtract,
                                                       Op.mult)
                        # next-region sigmoid plane + mean accum
                        if k < 3:
                            nc.scalar.activation(SGp[:, col(j)], Lj, SG_AF,
                                                 bias=sc(f"b{k + 2}"),
                                                 scale=float(SIG_AFF[k + 1][0]),
                                                 accum_out=accs[:, j:j + 1])
                        # P update
                        if k == 0:
                            nc.vector.tensor_scalar(Pb[:, col(j)], th[:, :], 1.0,
                                                    None, Op.add)
                        else:
                            nc.vector.scalar_tensor_tensor(Pb[:, col(j)], th[:, :],
                                                           1.0, Pb[:, col(j)],
                                                           Op.add, Op.mult)
                        # S update
                        if k == 1:
                            nc.vector.tensor_scalar(Sb[:, col(j)], th[:, :], 1.0,
                                                    1.0, Op.add, Op.min)
                        elif k >= 2:
                            u = ws.tile([P, F], b16, tag="w", name=f"u{k}_{j}_{rep}")
                            nc.vector.scalar_tensor_tensor(u[:, :], th[:, :], 1.0,
                                                           Sb[:, col(j)], Op.add,
                                                           Op.mult)
                            nc.vector.tensor_scalar(Sb[:, col(j)], u[:, :], 1.0,
                                                    None, Op.min)
                    if k < 3:
                        finish_mean(k + 1)
                stk2.close()
                stkp.close()
                stk.close()  # release Lp/SGp before final-phase planes

                # ---------------- final: b1 (x5 luma), b2 (tone If), b3 (apply)
                # Lp/SGp no longer needed; reuse their pool space via new tags
                # is not possible with bufs=1 pool; planes_pool already holds
                # them, so allocate L5/tr planes from a fresh pool.
                stkF = ExitStack()  # lives until end of b3
                fpl = stkF.enter_context(tc.tile_pool(name=f"fpl{rep}", bufs=1))
                L5b = fpl.tile([P, FREE], b16, tag="L5b")
                trb = fpl.tile([P, FREE], b16, tag="trb")
                stk3 = ExitStack()
                io1 = stk3.enter_context(tc.tile_pool(name=f"io1{rep}", bufs=6))
                ws1 = stk3.enter_context(tc.tile_pool(name=f"ws1{rep}", bufs=8))
                lt1 = stk3.enter_context(tc.tile_pool(name=f"lt1{rep}", bufs=2))
                for j in range(NCH):
                    r = io1.tile([P, F], f32, tag="i")
                    g = io1.tile([P, F], f32, tag="i")
                    b = io1.tile([P, F], f32, tag="i")
                    nc.sync.dma_start(r[:, :], img[0, :, col(j)])
                    nc.sync.dma_start(g[:, :], img[1, :, col(j)])
                    nc.sync.dma_start(b[:, :], img[2, :, col(j)])
                    x5 = []
                    for i_c, (tch, an) in enumerate(((r, "A_r"), (g, "A_g"),
                                                     (b, "A_b"))):
                        eng = nc.vector
                        eng.tensor_scalar(tch[:, :], tch[:, :], sc(an), sc("t"),
                                          Op.mult, Op.add)
                        x = ws1.tile([P, F], b16, tag="w", name=f"x5{i_c}_{j}_{rep}")
                        eng.tensor_scalar(x[:, :], tch[:, :], 0.0, sc("hi"),
                                          Op.max, Op.min)
                        eng.tensor_tensor(x[:, :], x[:, :], Pb[:, col(j)], Op.mult)
                        eng.tensor_tensor(x[:, :], x[:, :], Sb[:, col(j)], Op.min)
                        x5.append(x)
                    lt = lt1.tile([P, F], f32, tag="lt", name=f"lt{j}_{rep}")
                    nc.vector.tensor_scalar(lt[:, :], x5[0][:, :], 0.2126, None,
                                            Op.mult)
                    nc.vector.scalar_tensor_tensor(lt[:, :], x5[1][:, :], 0.7152,
                                                   lt[:, :], Op.mult, Op.add)
                    nc.vector.scalar_tensor_tensor(L5b[:, col(j)], x5[2][:, :],
                                                   0.0722, lt[:, :], Op.mult, Op.add)
                stk3.close()

                # b2: per-core tone curve ratio (8 If blocks, ACT only inside)
                stk4 = ExitStack()
                ws2 = stk4.enter_context(tc.tile_pool(name=f"ws2{rep}", bufs=4))
                for core in range(B):
                    with tc.If(pid == core):
                        for j in range(NCH):
                            nc.scalar.activation(trb[:, col(j)], L5b[:, col(j)],
                                                 TC_AF[core], scale=1023.0)
                # tr' = 1 + (L5 > 1e-5)*(tr - 1)
                for j in range(NCH):
                    msk = ws2.tile([P, F], b16, tag="m5", name=f"m5{j}_{rep}")
                    nc.vector.tensor_scalar(msk[:, :], L5b[:, col(j)], 1e-5, None,
                                            Op.is_gt)
                    nc.vector.scalar_tensor_tensor(trb[:, col(j)], trb[:, col(j)],
                                                   1.0, msk[:, :], Op.subtract,
                                                   Op.mult)
                    nc.vector.tensor_scalar(trb[:, col(j)], trb[:, col(j)], 1.0,
                                            None, Op.add)
                stk4.close()

                # b3: re-read, apply tone + vibrance + saturation, write out
                stk5 = ExitStack()
                io3 = stk5.enter_context(tc.tile_pool(name=f"io3{rep}", bufs=6))
                ws3 = stk5.enter_context(tc.tile_pool(name=f"ws3{rep}", bufs=14))
                lt3 = stk5.enter_context(tc.tile_pool(name=f"lt3{rep}", bufs=2))
                for j in range(NCH):
                    r = io3.tile([P, F], f32, tag="i")
                    g = io3.tile([P, F], f32, tag="i")
                    b = io3.tile([P, F], f32, tag="i")
                    nc.sync.dma_start(r[:, :], img[0, :, col(j)])
                    nc.sync.dma_start(g[:, :], img[1, :, col(j)])
                    nc.sync.dma_start(b[:, :], img[2, :, col(j)])
                    x6 = []
                    for i_c, (tch, an) in enumerate(((r, "A_r"), (g, "A_g"),
                                                     (b, "A_b"))):
                        eng = nc.vector
                        eng.tensor_scalar(tch[:, :], tch[:, :], sc(an), sc("t"),
                                          Op.mult, Op.add)
                        xb = ws3.tile([P, F], b16, tag="w", name=f"c{i_c}_{j}_{rep}")
                        eng.tensor_scalar(xb[:, :], tch[:, :], 0.0, sc("hi"),
                                          Op.max, Op.min)
                        eng.tensor_tensor(xb[:, :], xb[:, :], Pb[:, col(j)], Op.mult)
                        eng.tensor_tensor(xb[:, :], xb[:, :], Sb[:, col(j)], Op.min)
                        # tone curve apply: x6 = min(x5*tr', 1)
                        eng.tensor_tensor(xb[:, :], xb[:, :], trb[:, col(j)],
                                          Op.mult)
                        eng.tensor_scalar(xb[:, :], xb[:, :], 1.0, None, Op.min)
                        x6.append(xb)
                    lt = lt3.tile([P, F], b16, tag="lt", name=f"lt3{j}_{rep}")
                    Lv = ws3.tile([P, F], b16, tag="w", name=f"lv{j}_{rep}")
                    nc.vector.tensor_scalar(lt[:, :], x6[0][:, :], 0.2126, None,
                                            Op.mult)
                    nc.vector.scalar_tensor_tensor(lt[:, :], x6[1][:, :], 0.7152,
                                                   lt[:, :], Op.mult, Op.add)
                    nc.vector.scalar_tensor_tensor(Lv[:, :], x6[2][:, :], 0.0722,
                                                   lt[:, :], Op.mult, Op.add)
                    # chroma, chroma-norm^2
                    chs = []
                    ss = ws3.tile([P, F], b16, tag="w", name=f"ss{j}_{rep}")
                    s2 = ws3.tile([P, F], b16, tag="w", name=f"s2{j}_{rep}")
                    for i_c, x in enumerate(x6):
                        ch = ws3.tile([P, F], b16, tag="w", name=f"ch{i_c}_{j}_{rep}")
                        nc.vector.tensor_tensor(ch[:, :], x[:, :], Lv[:, :],
                                                Op.subtract)
                        chs.append(ch)
                    nc.vector.tensor_tensor(ss[:, :], chs[0][:, :], chs[0][:, :],
                                            Op.mult)
                    nc.vector.tensor_tensor(s2[:, :], chs[1][:, :], chs[1][:, :],
                                            Op.mult)
                    nc.vector.tensor_tensor(ss[:, :], ss[:, :], s2[:, :], Op.add)
                    nc.vector.tensor_tensor(s2[:, :], chs[2][:, :], chs[2][:, :],
                                            Op.mult)
                    nc.vector.tensor_tensor(ss[:, :], ss[:, :], s2[:, :], Op.add)
                    gn = ws3.tile([P, F], b16, tag="w", name=f"gn{j}_{rep}")
                    nc.scalar.activation(gn[:, :], ss[:, :], ES_AF, bias=sc("esb"))
                    nc.vector.tensor_scalar(gn[:, :], gn[:, :], sc("v"), 1.0,
                                            Op.mult, Op.add)
                    nc.vector.tensor_scalar(gn[:, :], gn[:, :], 4.0, 0.2,
                                            Op.min, Op.max)
                    # x7 = clip(Lv + ch*gn, 0, 1)
                    for i_c, ch in enumerate(chs):
                        eng = nc.vector
                        eng.tensor_tensor(ch[:, :], ch[:, :], gn[:, :], Op.mult)
                        eng.tensor_tensor(ch[:, :], ch[:, :], Lv[:, :], Op.add)
                        eng.tensor_scalar(ch[:, :], ch[:, :], 0.0, 1.0,
                                          Op.max, Op.min)
                    # saturation: out = clip(gs*x7 + (-sat/100)*lum(x7), 0, 1)
                    Ls = ws3.tile([P, F], b16, tag="w", name=f"ls{j}_{rep}")
                    nc.vector.tensor_scalar(lt[:, :], chs[0][:, :], 0.2126, None,
                                            Op.mult)
                    nc.vector.scalar_tensor_tensor(lt[:, :], chs[1][:, :], 0.7152,
                                                   lt[:, :], Op.mult, Op.add)
                    nc.vector.scalar_tensor_tensor(Ls[:, :], chs[2][:, :], 0.0722,
                                                   lt[:, :], Op.mult, Op.add)
                    Bs = ws3.tile([P, F], b16, tag="w", name=f"bs{j}_{rep}")
                    nc.vector.tensor_scalar(Bs[:, :], Ls[:, :], sc("omgs"), None,
                                            Op.mult)
                    for i_c, ch in enumerate(chs):
                        oc = io3.tile([P, F], b16, tag="o",
                                      name=f"oc{i_c}_{j}_{rep}")
                        nc.vector.scalar_tensor_tensor(oc[:, :], ch[:, :], sc("gs"),
                                                       Bs[:, :], Op.mult, Op.add)
                        nc.vector.tensor_scalar(oc[:, :], oc[:, :], 0.0, 1.0,
                                                Op.max, Op.min)
                        # fp16 -> f32 cast during the store (SWDGE)
                        nc.gpsimd.dma_start(out[i_c, :, col(j)], oc[:, :])
                stk5.close()
                stkF.close()

    import concourse.mybir as mybir2
    _split_drain_waits(nc, mybir2)
    return nc


# ----------------------------------------------------------------------------
# host side
# ----------------------------------------------------------------------------

def _host_params(inputs):
    def denorm(lo, hi, v):
        return lo + 0.5 * (v + 1.0) * (hi - lo)

    t64 = np.float64
    temp = denorm(2000.0, 50000.0, inputs["temperature_n"].astype(t64))
    tint = denorm(-150.0, 150.0, inputs["tint_n"].astype(t64))
    expo = denorm(-5.0, 5.0, inputs["exposure_n"].astype(t64))
    contr = denorm(-100.0, 100.0, inputs["contrast_n"].astype(t64))
    hl = denorm(-100.0, 100.0, inputs["highlights_n"].astype(t64))
    sh = denorm(-100.0, 100.0, inputs["shadows_n"].astype(t64))
    wh = denorm(-100.0, 100.0, inputs["whites_n"].astype(t64))
    bl = denorm(-100.0, 100.0, inputs["blacks_n"].astype(t64))
    vib = denorm(-100.0, 100.0, inputs["vibrance_n"].astype(t64))
    sat = denorm(-100.0, 100.0, inputs["saturation_n"].astype(t64))

    tr = 6500.0 / np.clip(temp, 2000.0, 50000.0)
    red = np.sqrt(tr)
    blue = 1.0 / np.sqrt(tr)
    ts = np.clip(tint / 150.0, -1.5, 1.5)
    green = 1.0 - 0.1 * ts
    red = red * (1.0 + 0.05 * ts)
    blue = blue * (1.0 - 0.05 * ts)
    gains = np.stack([red, green, blue], axis=1)  # [B,3]
    norm = np.maximum(gains.max(axis=1), 1e-4)
    G = gains / norm[:, None]
    e = np.power(2.0, expo)
    f = 1.0 + contr / 100.0
    A = G * (e * f)[:, None]
    t = 0.5 - 0.5 * f
    u = np.minimum(4.0 * e, 4.0)
    hi = np.clip(u * f + t, 0.0, 1.0)

    prm = np.zeros((B, NP_), np.float64)
    prm[:, PRM["A_r"]] = A[:, 0]
    prm[:, PRM["A_g"]] = A[:, 1]
    prm[:, PRM["A_b"]] = A[:, 2]
    prm[:, PRM["t"]] = t
    prm[:, PRM["hi"]] = hi
    prm[:, PRM["s1"]] = hl / 100.0
    prm[:, PRM["s2"]] = sh / 100.0
    prm[:, PRM["s3"]] = wh / 100.0
    prm[:, PRM["s4"]] = bl / 100.0
    prm[:, PRM["v"]] = vib / 100.0
    prm[:, PRM["gs"]] = 1.0 + sat / 100.0
    prm[:, PRM["omgs"]] = -sat / 100.0
    prm[:, PRM["invN"]] = 1.0 / NPIX
    for k in range(4):
        prm[:, PRM[f"b{k + 1}"]] = SIG_AFF[k][1] + 16.0
    prm[:, PRM["esb"]] = 1e-6
    return prm.astype(np.float32)


def _curves1024(tone_curve):
    c = tone_curve.astype(np.float64)  # [B,256]
    src = np.arange(1024) * (255.0 / 1023.0)
    i0 = np.floor(src).astype(int)
    i1 = np.minimum(i0 + 1, 255)
    w = src - i0
    return c[:, i0] * (1 - w) + c[:, i1] * w


_CACHE = {}
LAST_EXEC_NS = None


def _prepare(inputs):
    img = np.ascontiguousarray(inputs["image"], dtype=np.float32)
    curves = _curves1024(np.asarray(inputs["tone_curve"], np.float32))
    prm = _host_params({k: np.asarray(v, np.float32) for k, v in inputs.items()
                        if k != "image"})
    key = hashlib.sha256(curves.tobytes()).hexdigest()[:12]
    workdir = os.path.join(tempfile.gettempdir(), f"editv2_{key}")
    actroot = os.path.join(workdir, "actroot")
    if key not in _CACHE:
        os.makedirs(workdir, exist_ok=True)
        build_act_root(actroot, curves)
        os.environ["BASS_ACT_ROOT_JSON_PATH"] = os.path.join(actroot, "act_info.json")
        nc = build_kernel(key)
        _CACHE[key] = nc
    os.environ["BASS_ACT_ROOT_JSON_PATH"] = os.path.join(actroot, "act_info.json")
    in_maps = []
    for k in range(B):
        in_maps.append({
            f"img_{key}": img[k].reshape(C, P, FREE),
            "prm": np.broadcast_to(prm[k], (P, NP_)).copy(),
        })
    return _CACHE[key], in_maps, key


def kernel(**inputs):
    nc, in_maps, key = _prepare(inputs)
    from concourse.bass_utils import run_bass_kernel_spmd
    global LAST_EXEC_NS
    res = run_bass_kernel_spmd(nc, in_maps, core_ids=list(range(B)))
    if getattr(res, "exec_time_ns", None):
        LAST_EXEC_NS = res.exec_time_ns
    outs = [res.results[k]["out"].reshape(C, H, W) for k in range(B)]
    return np.stack(outs, axis=0)


# ----------------------------------------------------------------------------
# on-device execution-time measurement
#
# This container has no working NTFF/neuron-profile hook (the axon profiling
# hook is absent), and a single dispatch is dominated by a fixed ~83 ms
# PJRT-over-axon round-trip that is independent of the kernel (measured: a
# 8 KB copy NEFF and a 37 MB kernel both take ~84 ms wall).  To measure the
# actual on-device execution time we build the same kernel with its body
# repeated R times inside one NEFF and take the slope:
#     t_exec = (T(R2) - T(R1)) / (R2 - R1)
# which cancels the dispatch floor exactly.  Inputs are staged on device
# once; each timed call re-supplies fresh donated output buffers.
# ----------------------------------------------------------------------------

def _make_runner(nc):
    import jax
    import jax.numpy as jnp
    from jax.sharding import Mesh, PartitionSpec, NamedSharding
    from jax.experimental.shard_map import shard_map
    import concourse.bass2jax as b2j
    import concourse.mybir as mybir

    b2j.install_neuronx_cc_hook()
    partition_name = nc.partition_id_tensor.name if nc.partition_id_tensor else None
    in_names, out_names, out_avals, zero_shapes = [], [], [], []
    for alloc in nc.m.functions[0].allocations:
        if not isinstance(alloc, mybir.MemoryLocationSet):
            continue
        name = alloc.memorylocations[0].name
        if alloc.kind == "ExternalInput":
            if name != partition_name:
                in_names.append(name)
        elif alloc.kind == "ExternalOutput":
            out_names.append(name)
            shape = tuple(alloc.tensor_shape)
            dty = mybir.dt.np(alloc.dtype)
            out_avals.append(jax.core.ShapedArray(shape, dty))
            zero_shapes.append((shape, dty))
    n_params = len(in_names)
    n_outs = len(out_avals)
    all_in_names = list(in_names) + list(out_names)
    if partition_name is not None:
        all_in_names.append(partition_name)

    def _body(*args):
        operands = list(args)
        if partition_name is not None:
            operands.append(b2j.partition_id_tensor())
        outs = b2j._bass_exec_p.bind(
            *operands, out_avals=tuple(out_avals),
            in_names=tuple(all_in_names), out_names=tuple(out_names),
            lowering_input_output_aliases=(),
            sim_require_finite=True, sim_require_nnan=True, nc=nc)
        return tuple(outs)

    donate = tuple(range(n_params, n_params + n_outs))
    devices = jax.devices()[:B]
    mesh = Mesh(np.asarray(devices), ("core",))
    sharded = jax.jit(
        shard_map(_body, mesh=mesh,
                  in_specs=(PartitionSpec("core"),) * (n_params + n_outs),
                  out_specs=(PartitionSpec("core"),) * n_outs,
                  check_rep=False),
        donate_argnums=donate, keep_unused=True)
    sh = NamedSharding(mesh, PartitionSpec("core"))
    return sharded, in_names, zero_shapes, sh


def _bench_nc(nc, in_maps, n_iter=10, dev_cache={}):
    import time
    import jax
    import jax.numpy as jnp
    sharded, in_names, zero_shapes, sh = _make_runner(nc)
    ck = tuple(in_names)
    if ck not in dev_cache:
        concat_in = [
            np.concatenate([np.asarray(in_maps[c][nm]) for c in range(B)], axis=0)
            for nm in in_names]
        dev_in = [jax.device_put(a, sh) for a in concat_in]
        jax.block_until_ready(dev_in)
        dev_cache[ck] = dev_in
    dev_in = dev_cache[ck]

    def make_zeros():
        zs = [jax.device_put(jnp.zeros((B * s[0], *s[1:]), dty), sh)
              for (s, dty) in zero_shapes]
        jax.block_until_ready(zs)
        return zs

    zs = make_zeros()
    out = sharded(*dev_in, *zs)
    jax.block_until_ready(out)   # compile + warm
    times = []
    for _ in range(n_iter):
        zs = make_zeros()
        t0 = time.time()
        out = sharded(*dev_in, *zs)
        jax.block_until_ready(out)
        times.append(time.time() - t0)
    return min(times), out


def _bench_pair(nc1, nc2, in_maps, n_pairs=12):
    """Interleaved timing of two NEFF variants: alternating calls cancel the
    slow drift of the PJRT-over-axon dispatch floor; returns the median of
    per-pair wall-time differences."""
    import time
    import jax
    import jax.numpy as jnp
    r1 = _make_runner(nc1)
    r2 = _make_runner(nc2)
    sharded1, in_names, zero_shapes, sh = r1
    sharded2 = r2[0]
    concat_in = [
        np.concatenate([np.asarray(in_maps[c][nm]) for c in range(B)], axis=0)
        for nm in in_names]
    dev_in = [jax.device_put(a, sh) for a in concat_in]
    jax.block_until_ready(dev_in)

    def make_zeros():
        zs = [jax.device_put(jnp.zeros((B * s[0], *s[1:]), dty), sh)
              for (s, dty) in zero_shapes]
        jax.block_until_ready(zs)
        return zs

    for f in (sharded1, sharded2):  # compile + warm both
        out = f(*dev_in, *make_zeros())
        jax.block_until_ready(out)
    diffs, t1s, t2s = [], [], []
    for _ in range(n_pairs):
        zs = make_zeros()
        t0 = time.time()
        out = sharded1(*dev_in, *zs)
        jax.block_until_ready(out)
        t1 = time.time() - t0
        zs = make_zeros()
        t0 = time.time()
        out = sharded2(*dev_in, *zs)
        jax.block_until_ready(out)
        t2 = time.time() - t0
        t1s.append(t1); t2s.append(t2); diffs.append(t2 - t1)
    diffs.sort()
    med = diffs[len(diffs) // 2]
    return med, min(t1s), min(t2s)


def measure_hw_exec_ns(inputs, r1=1, r2=9, n_pairs=12):
    """Returns (per_exec_ns, details). Two NEFFs with the kernel body
    repeated r1/r2 times; per-execution on-device time = the median
    interleaved-pair difference divided by (r2 - r1), which cancels the
    ~83 ms kernel-independent dispatch round-trip exactly."""
    nc1, in_maps, key = _prepare(inputs)
    if r1 != 1:
        nc1 = build_kernel(key, repeat=r1)
    nc2 = build_kernel(key, repeat=r2)
    med_diff, t1, t2 = _bench_pair(nc1, nc2, in_maps, n_pairs)
    per = max(med_diff / (r2 - r1), 1e-6)
    return int(per * 1e9), {"T_r1_ms": t1 * 1e3, "T_r2_ms": t2 * 1e3,
                            "r1": r1, "r2": r2}


if __name__ == "__main__":
    import reference
    inputs = {k: np.asarray(v) for k, v in reference.setup_inputs().items()}
    outp = kernel(**inputs)
    exp = np.asarray(reference.reference(**inputs))
    err = np.abs(outp - exp)
    print("max abs err:", err.max(), "mean:", err.mean())
